# revision 48
# baseline (speedup 1.0000x reference)
"""Trainium2 Bass kernel for relu-kernelized multi-head attention with a
per-head Toeplitz relative-position mask (sparse_attention problem).

Contract: kernel(**inputs) takes FULL unsharded inputs (numpy), returns the
FULL output [16, 1025, 768]. Internally: data-parallel over batch across 8
NeuronCores (2 batches/core), identical SPMD program, per-core inputs differ
only in the x shard.

Math (per batch b):
  q = relu((x@wq + bq)/8) + eps ; k = relu(x@wk + bk) + eps ; v = x@wv + bv
  S[q,k] = sum_d q*k ;  attn = S*|tm| + eps ; attn /= rowsum ; out = attn@v
  y = out@wo + bo

Perf structure (v6):
  - all matmul operands bf16 (PE 1 cycle/row vs fp32's 4), fp32 PSUM.
  - every logical load is ONE DMA: host pre-packs all tensors in the exact
    [partition, ...] SBUF layout (DMA issue on the sync queue costs ~650ns
    each - the v2 kernel spent >160us there).
  - S/AV j-loop is software-pipelined (AV_j after S_{j+1}) so the PE never
    waits on the DVE mask-multiply.
  - row-normalization batched per head pair: one [4,L] reciprocal, DMA
    partition-broadcast of 1/r via a DRAM bounce, all on the gpsimd queue.
  - attention outputs stay in SBUF as 12 [128,L] bf16 head-pair tiles
    consumed directly by the O projection; output shipped bf16.
  - the q/k "+eps" of the reference is dropped (~1e-7 relative effect); the
    attention-level eps is kept via the cs rank-1 correction and the
    rowsum + L*eps denominator.
"""

import os
import sys

sys.path.insert(0, "/opt/trn_rl_repo")

import numpy as np

B, L, F, H, D = 16, 1025, 768, 12, 64
NB = 32
EPS = 1e-8
LP = 1152           # padded token count (9 * 128)
NKB = 9             # k blocks of 128
QM = 1024           # main q width (q tail = 1 col, index 1024)
FA = F + 1          # augmented contraction (ones row)
NCORES = 8
BPC = B // NCORES   # batches per core

_PROG = None


def _build_program():
    import concourse.bass as bass
    import concourse.tile as tile
    from concourse import mybir

    f32 = mybir.dt.float32
    bf16 = mybir.dt.bfloat16
    AF = mybir.ActivationFunctionType

    nc = bass.Bass()

    xaP = nc.declare_dram_parameter("xaP", [BPC, 128, 6, LP], bf16, isOutput=False)
    wqP = nc.declare_dram_parameter("wqP", [6, 128, 6, 128], bf16, isOutput=False)
    wkP = nc.declare_dram_parameter("wkP", [6, 128, 6, 128], bf16, isOutput=False)
    wvP = nc.declare_dram_parameter("wvP", [2, 128, 7, 390], bf16, isOutput=False)
    woP = nc.declare_dram_parameter("woP", [128, 6, F], bf16, isOutput=False)
    boP = nc.declare_dram_parameter("boP", [128, 6], f32, isOutput=False)
    bqkP = nc.declare_dram_parameter("bqkP", [128, 12], f32, isOutput=False)
    csP = nc.declare_dram_parameter("csP", [65, 24], f32, isOutput=False)
    maskP = nc.declare_dram_parameter(
        "maskP", [H, 128, NKB, QM], bf16, isOutput=False
    )
    mask_tail = nc.declare_dram_parameter(
        "maskT_tail", [H, 128, NKB], bf16, isOutput=False
    )
    yT = nc.declare_dram_parameter("yT", [BPC, 128, 6, L], bf16, isOutput=True)

    # rows padded to LP so [128, 9] partition-transposed reads/writes of the
    # 1025 live elements stay in-bounds
    rr_dram = nc.dram_tensor("rr_dram", [8, LP], f32)
    rsum_dram = nc.dram_tensor("rsum_dram", [8, LP], f32)

    with tile.TileContext(nc) as tc:
        from contextlib import ExitStack

        with ExitStack() as octx:
            consts = octx.enter_context(tc.tile_pool(name="consts", bufs=1))
            # attention outputs, SBUF-resident across phases: 12 tiles
            # [128, L] bf16, one per (batch, head-pair); rows 0:64 = even
            # head, 64:128 = odd head of the pair
            ot_pool = octx.enter_context(tc.tile_pool(name="ot", bufs=2 * 6))
            wo_pool = octx.enter_context(tc.tile_pool(name="wo", bufs=1))
            ctx = octx.enter_context(ExitStack())
            xa_pool = ctx.enter_context(tc.tile_pool(name="xa", bufs=2))
            wqk_pool = ctx.enter_context(tc.tile_pool(name="wqk", bufs=2))
            wv_pool = ctx.enter_context(tc.tile_pool(name="wv", bufs=2))
            qkt_pool = ctx.enter_context(tc.tile_pool(name="qkt", bufs=2))
            vaug_pool = ctx.enter_context(tc.tile_pool(name="vaug", bufs=4))
            mask_pool = ctx.enter_context(tc.tile_pool(name="mask", bufs=2))
            mtail_pool = ctx.enter_context(tc.tile_pool(name="mtail", bufs=2))
            mt_pool = ctx.enter_context(tc.tile_pool(name="mt", bufs=4))
            std_pool = ctx.enter_context(tc.tile_pool(name="std", bufs=2))
            mttail_pool = ctx.enter_context(tc.tile_pool(name="mttail", bufs=2))
            rs_pool = ctx.enter_context(tc.tile_pool(name="rs", bufs=1))
            rrb_pool = ctx.enter_context(tc.tile_pool(name="rrb", bufs=3))
            avsb_pool = ctx.enter_context(tc.tile_pool(name="avsb", bufs=4))

            # flex pool: [128,512] tiles time-shared between projection psums
            # (2-deep so the activation drain doesn't stall the next matmul
            # group) and the per-head tail psum (stail+avt live in a slice)
            ps_flex = ctx.enter_context(
                tc.tile_pool(name="ps_flex", bufs=2, space="PSUM")
            )
            ps_s = ctx.enter_context(tc.tile_pool(name="ps_s", bufs=2, space="PSUM"))
            ps_av = ctx.enter_context(tc.tile_pool(name="ps_av", bufs=1, space="PSUM"))

            # three DMA-issue queues: sync carries x/weights/outputs, the
            # scalar queue carries the big mask transfers, gpsimd carries
            # the normalize path + upfront small loads
            dma = nc.sync
            dma2 = nc.gpsimd
            dma3 = nc.scalar

            # constants
            ones_row = consts.tile([1, LP], bf16)
            nc.vector.memset(ones_row[:, 0:L], 1.0)
            nc.vector.memset(ones_row[:, L:LP], 0.0)
            bq_all = consts.tile([128, 12], f32, name="bq_all")
            dma2.dma_start(out=bq_all, in_=bqkP[:, :])
            cs_all = consts.tile([65, 24], f32, name="cs_all")
            dma2.dma_start(out=cs_all, in_=csP[:, :])
            bo_sb = consts.tile([128, 6], f32, name="bo_sb")
            dma2.dma_start(out=bo_sb, in_=boP[:, :])

            ot_pairs = {}
            for b in range(BPC):
                for pair in range(6):
                    ot_pairs[(b, pair)] = ot_pool.tile(
                        [128, L], bf16, tag="ot", name="ot_pair"
                    )

            # ---- persistent x in SBUF: one [128, 6, LP] tile per batch ---
            # per-chunk DMAs so the first V-proj matmul starts as soon as
            # chunk 0 lands instead of waiting for the whole 1.8MB tile
            xa_t = {}
            for b in range(BPC):
                t = xa_pool.tile([128, 6, LP], bf16, tag="xa", name="xa_tile")
                for c in range(6):
                    dma.dma_start(
                        out=t[:, c : c + 1, :], in_=xaP[b, :, c : c + 1, :]
                    )
                xa_t[b] = t

            # output-projection weights, prefetched so the O phase starts
            # without a DMA stall
            wo_sb = wo_pool.tile([128, 6, F], bf16, name="wo_sb")
            dma2.dma_start(out=wo_sb, in_=woP[:, :, :])

            # q sub-tiles for projections (moving dim <= 512); only token
            # 1024 of the padded tail is real
            qsubs = [(0, 512), (512, 512), (1024, 1)]
            # attention q tiling: main [0,1024) in 2 psum-bank halves + tail col
            def st_slices():
                return [(0, 512), (512, 512)]

            # ---- v projections, per 3-pair group ------------------------
            # wv columns are grouped per head: h*65 + (0..63 -> wv, 64 -> ones)
            vaug = {}      # (b, g) -> [128, NKB, 390]

            wv_tiles = {}

            def emit_vproj(g, b):
                if g not in wv_tiles:
                    wv_sb = wv_pool.tile([128, 7, 390], bf16, tag="wv")
                    for c in range(7):
                        dma3.dma_start(
                            out=wv_sb[:, c : c + 1, :], in_=wvP[g, :, c : c + 1, :]
                        )
                    wv_tiles[g] = wv_sb
                wv_sb = wv_tiles[g]
                va = vaug_pool.tile([128, NKB, 390], bf16, tag="vaug")
                for tb in range(NKB):
                    ps = ps_flex.tile([128, 512], f32, tag="flex", name="ps_v")
                    for c in range(6):
                        nc.tensor.matmul(
                            ps[:, 0:390],
                            xa_t[b][:, c, tb * 128 : (tb + 1) * 128],
                            wv_sb[:, c, :],
                            start=(c == 0),
                            stop=False,
                        )
                    nc.tensor.matmul(
                        ps[:, 0:390],
                        ones_row[:, tb * 128 : (tb + 1) * 128],
                        wv_sb[0:1, 6, :],
                        start=False,
                        stop=True,
                    )
                    nc.scalar.activation(va[:, tb, :], ps[:, 0:390], AF.Copy)
                vaug[(b, g)] = va

            # ---- main loop over head pairs ------------------------------
            for pair in range(6):
                g = pair // 3

                # qT/kT projections for this pair, both batches
                wq_sb = wqk_pool.tile([128, 6, 128], bf16, tag="wq")
                wk_sb = wqk_pool.tile([128, 6, 128], bf16, tag="wk")
                dma.dma_start(out=wq_sb, in_=wqP[pair])
                dma.dma_start(out=wk_sb, in_=wkP[pair])

                qT = {}
                kT = {}

                def qk_proj(b):
                    qt = qkt_pool.tile([128, LP], bf16, tag="qT")
                    kt = qkt_pool.tile([128, LP], bf16, tag="kT")
                    # k-pad columns are read by the j=8 S matmul (masked to
                    # zero afterwards) - keep them finite
                    nc.vector.memset(kt[:, L:LP], 0.0)
                    for (dst, w_sb, scl, bi) in (
                        (qt, wq_sb, 0.125, 0),
                        (kt, wk_sb, 1.0, 1),
                    ):
                        for (q0, qw) in qsubs:
                            psq = ps_flex.tile(
                                [128, 512], f32, tag="flex", name="ps_qk"
                            )
                            for c in range(6):
                                nc.tensor.matmul(
                                    psq[:, 0:qw],
                                    w_sb[:, c, :],
                                    xa_t[b][:, c, q0 : q0 + qw],
                                    start=(c == 0), stop=(c == 5),
                                )
                            # relu(scale*xw + scale*b); the reference's +eps
                            # here is dropped (~1e-7 relative effect)
                            nc.scalar.activation(
                                dst[:, q0 : q0 + qw], psq[:, 0:qw], AF.Relu,
                                scale=scl,
                                bias=bq_all[:, 2 * pair + bi : 2 * pair + bi + 1],
                            )
                    qT[b] = qt
                    kT[b] = kt

                if pair == 0:
                    # interleave so b=0 compute overlaps the arrival of
                    # batch 1's x DMA at kernel start
                    emit_vproj(g, 0)
                    qk_proj(0)
                    emit_vproj(g, 1)
                    qk_proj(1)
                else:
                    if pair % 3 == 0:
                        emit_vproj(g, 0)
                        emit_vproj(g, 1)
                    qk_proj(0)
                    qk_proj(1)

                av_sbs = {}
                for hh in range(2):
                    h = pair * 2 + hh
                    r0 = hh * 64
                    # mask tile for this head (shared across batches)
                    mk = mask_pool.tile(
                        [128, NKB, QM], bf16, tag="mask", name="mask_tile"
                    )
                    dma3.dma_start(out=mk, in_=maskP[h])
                    mkt = mtail_pool.tile([128, NKB], bf16, tag="mtail")
                    dma3.dma_start(out=mkt, in_=mask_tail[h])

                    for b in range(BPC):
                        va = vaug[(b, pair // 3)]
                        vc0 = (pair % 3) * 130 + hh * 65

                        av = ps_av.tile([65, QM], f32, tag="ps_av")
                        ptl = ps_flex.tile(
                            [128, 512], f32, tag="flex", name="ps_tails"
                        )
                        stail = ptl[:, 0:NKB]
                        avt = ptl[0:65, NKB : NKB + 1]
                        mtt = mttail_pool.tile([128, NKB], bf16, tag="mttail")

                        # software-pipelined at depth 2: AV_j issues after
                        # S_{j+2}, so the PE never waits on the scalar
                        # drain + DVE 2x-mode mask-multiply chain
                        def emit_s(j):
                            lhs_k = kT[b][r0 : r0 + 64, j * 128 : (j + 1) * 128]
                            st = ps_s.tile([128, QM], f32, tag="ps_s")
                            for (q0, qw) in st_slices():
                                nc.tensor.matmul(
                                    st[:, q0 : q0 + qw],
                                    lhs_k,
                                    qT[b][r0 : r0 + 64, q0 : q0 + qw],
                                    start=True, stop=True,
                                )
                            # tail column q=1024 (shares the kT weights)
                            nc.tensor.matmul(
                                stail[:, j : j + 1],
                                lhs_k,
                                qT[b][r0 : r0 + 64, QM : QM + 1],
                                start=True, stop=True,
                            )
                            # masked scores -> bf16.  Alternate per j: even
                            # j drains to bf16 on the scalar engine so the
                            # DVE multiply runs in 2x mode; odd j multiplies
                            # straight from PSUM at 1x.  This splits the
                            # ~1.1us/j chain across both engines so neither
                            # becomes the S-loop rate limiter.
                            mt = mt_pool.tile([128, QM], bf16, tag="mt")
                            if j % 2 == 0:
                                std = std_pool.tile([128, QM], bf16, tag="std")
                                nc.scalar.activation(std, st, AF.Copy)
                                nc.vector.tensor_mul(mt, std, mk[:, j, :])
                            else:
                                nc.vector.tensor_mul(mt, st, mk[:, j, :])
                            return mt

                        def emit_av(j, mt):
                            # AV accumulation (row 64 = rowsum via ones col)
                            for (q0, qw) in st_slices():
                                nc.tensor.matmul(
                                    av[:, q0 : q0 + qw],
                                    va[:, j, vc0 : vc0 + 65],
                                    mt[:, q0 : q0 + qw],
                                    start=(j == 0), stop=(j == NKB - 1),
                                )

                        mts = [emit_s(0), emit_s(1)]
                        for j in range(2, NKB):
                            mts.append(emit_s(j))
                            emit_av(j - 2, mts[j - 2])
                        emit_av(NKB - 2, mts[NKB - 2])
                        emit_av(NKB - 1, mts[NKB - 1])

                        # tail: masked scores + AV
                        nc.vector.tensor_mul(mtt, stail, mkt)
                        for j in range(NKB):
                            nc.tensor.matmul(
                                avt,
                                va[:, j, vc0 : vc0 + 65],
                                mtt[:, j : j + 1],
                                start=(j == 0), stop=(j == NKB - 1),
                            )

                        # drain AV psum to SBUF (frees the banks for the
                        # next head while the normalize chain runs)
                        av_sb = avsb_pool.tile([65, L], f32, tag="avsb")
                        nc.scalar.activation(av_sb[:, 0:QM], av, AF.Copy)
                        nc.scalar.activation(av_sb[:, QM : QM + 1], avt, AF.Copy)
                        av_sbs[(hh, b)] = av_sb

                # ---- batched normalization for the pair's 4 (hh, b) -----
                # gather rowsum rows TRANSPOSED across partitions so the
                # reciprocal runs ~36 elems/lane (~0.3us) instead of 1025
                # serial elems on 4 lanes (~6.5us blocking the DVE queue)
                def normalize(combos, slot0):
                    n = len(combos)
                    rs = rs_pool.tile([128, 4 * NKB], f32, tag="rs")
                    for idx, (hh, b) in enumerate(combos):
                        # bounce the rowsum row through DRAM, reading it
                        # back transposed across partitions:
                        # rs[p, idx*9 + c] = rowsum[c*128 + p]
                        slot = slot0 + idx
                        dma2.dma_start(
                            out=rsum_dram[slot, 0:L],
                            in_=av_sbs[(hh, b)][64:65, :],
                        )
                        base = rsum_dram[slot]
                        src = bass.AP(
                            tensor=base.tensor,
                            offset=base.offset,
                            ap=[[1, 128], [128, NKB]],
                        )
                        dma2.dma_start(
                            out=rs[:, idx * NKB : (idx + 1) * NKB], in_=src
                        )
                    nc.vector.tensor_scalar_add(
                        rs[:, 0 : n * NKB], rs[:, 0 : n * NKB], float(L) * EPS
                    )
                    rr = rs_pool.tile([128, 4 * NKB], f32, tag="rr")
                    nc.vector.reciprocal(rr[:, 0 : n * NKB], rs[:, 0 : n * NKB])
                    # scatter back to flat rows in DRAM for the broadcast
                    for idx in range(n):
                        base = rr_dram[slot0 + idx]
                        dst = bass.AP(
                            tensor=base.tensor,
                            offset=base.offset,
                            ap=[[1, 128], [128, NKB]],
                        )
                        dma2.dma_start(
                            out=dst, in_=rr[:, idx * NKB : (idx + 1) * NKB]
                        )
                    for idx, (hh, b) in enumerate(combos):
                        rr_slot = rr_dram[slot0 + idx, 0:L]
                        rr_bcast_src = bass.AP(
                            tensor=rr_slot.tensor,
                            offset=rr_slot.offset,
                            ap=[[0, 64]] + list(rr_slot.ap),
                        )
                        rrb = rrb_pool.tile([64, L], f32, tag="rrb")
                        dma2.dma_start(out=rrb, in_=rr_bcast_src)
                        hg = (pair % 3) * 2 + hh
                        ci = b * 12 + g * 6 + hg
                        r0h = hh * 64
                        nc.vector.scalar_tensor_tensor(
                            ot_pairs[(b, pair)][r0h : r0h + 64, :],
                            av_sbs[(hh, b)][0:64, :],
                            cs_all[0:64, ci : ci + 1],
                            rrb,
                            op0=mybir.AluOpType.add,
                            op1=mybir.AluOpType.mult,
                        )

                if pair < 5:
                    normalize(
                        [(hh, b) for hh in range(2) for b in range(BPC)],
                        (pair % 2) * 4,
                    )
                else:
                    # last pair: per-batch so the O projection of b=0 isn't
                    # gated on b=1's normalize chain
                    normalize([(0, 0), (1, 0)], 4)
                    normalize([(0, 1), (1, 1)], 6)

            # ---- output projection: yT = wo^T @ O^T + bo ----------------
            ctx.close()
            ctx = octx.enter_context(ExitStack())
            y_pool = ctx.enter_context(tc.tile_pool(name="y", bufs=3))
            ps_y = ctx.enter_context(tc.tile_pool(name="ps_y", bufs=2, space="PSUM"))

            oq_tiles = [(0, 512), (512, 512), (1024, 1)]
            for b in range(BPC):
                for fc in range(6):
                    y_tile = y_pool.tile([128, L], bf16, tag="y", name="y_tile")
                    for (q0, qw) in oq_tiles:
                        psy = ps_y.tile([128, 512], f32, tag="ps_y")
                        for hc in range(6):
                            nc.tensor.matmul(
                                psy[:, 0:qw],
                                wo_sb[:, hc, fc * 128 : (fc + 1) * 128],
                                ot_pairs[(b, hc)][:, q0 : q0 + qw],
                                start=(hc == 0), stop=(hc == 5),
                            )
                        # drain with bo fused as the per-partition bias
                        nc.scalar.activation(
                            y_tile[:, q0 : q0 + qw], psy[:, 0:qw],
                            AF.Identity, bias=bo_sb[:, fc : fc + 1],
                        )
                    dma.dma_start(out=yT[b][:, fc, :], in_=y_tile)

    _split_matmul_waits(nc)
    return nc


def _split_matmul_waits(nc):
    """Walrus TPB instruction structs encode a limited number of sync waits
    (the fp32 LDWEIGHTS+MATMUL pair can take none beyond its update).  Hoist
    excess waits onto same-engine NoOps inserted just before each
    instruction."""
    import bass_rust
    from concourse import mybir

    n = 0
    for f in nc.m.functions:
        for blk in f.blocks:
            insts = blk.instructions
            out = []
            for inst in insts:
                si = inst.sync_info
                tname = type(inst).__name__
                if si is not None and len(si.on_wait) > 0 and tname != "InstISA":
                    cap = 0 if tname == "InstMatmult" else 1
                    waits = list(si.on_wait)
                    if len(waits) > cap:
                        hoist = waits[: len(waits) - cap]
                        keep = waits[len(waits) - cap :]
                        for w in hoist:
                            nop = mybir.InstNoOp(
                                name=f"I-mmw-{n}", ins=[], outs=[]
                            )
                            n += 1
                            nop.engine = inst.engine
                            nop.sync_info = bass_rust.SyncInfo(
                                on_wait=[w], on_update=[]
                            )
                            out.append(nop)
                        inst.sync_info = bass_rust.SyncInfo(
                            on_wait=keep, on_update=list(si.on_update)
                        )
                out.append(inst)
            insts[:] = out
    return n


def _dist_index():
    gi = np.arange(NB)
    gj = np.arange(NB)
    idx = (
        (gi[:, None, None, None] - gi[None, None, :, None] + NB) * 2 * NB
        + gj[None, :, None, None]
        - gj[None, None, None, :]
        + NB
    )
    return idx.reshape(-1).astype(np.int32)


def _host_prep(x, wq, bq, wk, bk, wv, bv, wo, bo, toeplitz_params):
    import ml_dtypes

    f4 = np.float32
    bf = ml_dtypes.bfloat16
    x = np.asarray(x, f4)
    L0 = NB * NB

    # x, transposed to [F, L], padded to LP, packed [128, 6, LP]
    xs = np.transpose(x, (0, 2, 1))  # [B, F, L]
    xaP = np.zeros((B, 128, 6, LP), bf)
    xaP[:, :, :, :L] = xs.reshape(B, 6, 128, L).transpose(0, 2, 1, 3).astype(bf)

    wq_flat = np.asarray(wq, f4).reshape(F, F)
    wk_flat = np.asarray(wk, f4).reshape(F, F)
    wqP = np.ascontiguousarray(
        wq_flat.reshape(6, 128, 6, 128).transpose(2, 1, 0, 3).astype(bf)
    )
    wkP = np.ascontiguousarray(
        wk_flat.reshape(6, 128, 6, 128).transpose(2, 1, 0, 3).astype(bf)
    )

    wvr = np.asarray(wv, f4)
    bvr = np.asarray(bv, f4)
    wv_aug = np.zeros((FA, H * 65), f4)
    for h in range(H):
        wv_aug[:F, h * 65 : h * 65 + 64] = wvr[:, h, :]
        wv_aug[F, h * 65 : h * 65 + 64] = bvr[h]
        wv_aug[F, h * 65 + 64] = 1.0
    wvP = np.zeros((2, 128, 7, 390), bf)
    wvP[:, :, :6, :] = (
        wv_aug[:F].reshape(6, 128, 2, 390).transpose(2, 1, 0, 3).astype(bf)
    )
    wvP[:, 0, 6, :] = wv_aug[F].reshape(2, 390).astype(bf)

    wo_flat = np.asarray(wo, f4).reshape(H * D, F)
    woP = np.ascontiguousarray(
        wo_flat.reshape(6, 128, F).transpose(1, 0, 2).astype(bf)
    )
    boP = np.ascontiguousarray(np.asarray(bo, f4).reshape(6, 128).T)

    bqs = (np.asarray(bq, f4).reshape(F) * 0.125).reshape(6, 128)
    bks = np.asarray(bk, f4).reshape(F).reshape(6, 128)
    bqkP = np.zeros((128, 12), f4)
    bqkP[:, 0::2] = bqs.T
    bqkP[:, 1::2] = bks.T

    # gathered |toeplitz| mask, padded (CLS row/col of ones), transposed,
    # k padded to 1152 with zeros, packed [H, 128, NKB, QM]
    tp = np.asarray(toeplitz_params, f4)
    tm = np.abs(tp[:, _dist_index()]).reshape(H, L0, L0)
    tm_full = np.ones((H, L, L), f4)
    tm_full[:, 1:, 1:] = tm
    maskT = np.zeros((H, LP, L), bf)
    maskT[:, :L, :] = np.transpose(tm_full, (0, 2, 1)).astype(bf)
    maskP = np.ascontiguousarray(
        maskT[:, :, :QM].reshape(H, NKB, 128, QM).transpose(0, 2, 1, 3)
    )
    maskT_tail = np.ascontiguousarray(
        maskT[:, :, QM].reshape(H, NKB, 128).transpose(0, 2, 1)
    )

    xsum = x.sum(axis=1)  # [B, F]
    cs = np.einsum("bf,fhd->bhd", xsum, wvr) + L * bvr[None]  # [B, H, 64]
    cs_full = np.concatenate(
        [cs, np.full((B, H, 1), float(L), np.float32)], axis=2
    ) * np.float32(EPS)  # [B, H, 65]

    shared = dict(
        bqkP=bqkP,
        wqP=wqP,
        wkP=wkP,
        wvP=wvP,
        woP=woP,
        boP=boP,
        maskP=maskP,
        maskT_tail=maskT_tail,
    )
    in_maps = []
    for c in range(NCORES):
        m = dict(shared)
        m["xaP"] = np.ascontiguousarray(xaP[c * BPC : (c + 1) * BPC])
        csP = np.zeros((65, 24), f4)
        for b in range(BPC):
            for g in range(2):
                for hg in range(6):
                    csP[:, b * 12 + g * 6 + hg] = cs_full[
                        c * BPC + b, 6 * g + hg, :
                    ]
        m["csP"] = csP
        in_maps.append(m)
    return in_maps


def _get_program():
    global _PROG
    if _PROG is None:
        _PROG = _build_program()
    return _PROG


def run(trace=False, **inputs):
    from concourse.bass_utils import run_bass_kernel_spmd

    nc = _get_program()
    in_maps = _host_prep(**inputs)
    res = run_bass_kernel_spmd(nc, in_maps, list(range(NCORES)), trace=trace)
    outs = []
    for c in range(NCORES):
        yt = np.asarray(res.results[c]["yT"], dtype=np.float32)  # [BPC,128,6,L]
        # y[b, l, fc*128 + p] = yt[b, p, fc, l]
        outs.append(yt.transpose(0, 3, 2, 1).reshape(BPC, L, F))
    y = np.concatenate(outs, axis=0).astype(np.float32)
    return y, res


def kernel(**inputs):
    y, _ = run(trace=False, **inputs)
    return y


# revision 50
# speedup vs baseline: 1.4877x; 1.4877x over previous
"""Trainium2 Bass kernel for relu-kernelized multi-head attention with a
per-head Toeplitz relative-position mask (sparse_attention problem).

Contract: kernel(**inputs) takes FULL unsharded inputs (numpy), returns the
FULL output [16, 1025, 768]. Internally: data-parallel over batch across 8
NeuronCores (2 batches/core), identical SPMD program, per-core inputs differ
only in the x shard.

Math (per batch b):
  q = relu((x@wq + bq)/8) + eps ; k = relu(x@wk + bk) + eps ; v = x@wv + bv
  S[q,k] = sum_d q*k ;  attn = S*|tm| + eps ; attn /= rowsum ; out = attn@v
  y = out@wo + bo

Perf structure (v6):
  - all matmul operands bf16 (PE 1 cycle/row vs fp32's 4), fp32 PSUM.
  - every logical load is ONE DMA: host pre-packs all tensors in the exact
    [partition, ...] SBUF layout (DMA issue on the sync queue costs ~650ns
    each - the v2 kernel spent >160us there).
  - S/AV j-loop is software-pipelined (AV_j after S_{j+1}) so the PE never
    waits on the DVE mask-multiply.
  - row-normalization batched per head pair: one [4,L] reciprocal, DMA
    partition-broadcast of 1/r via a DRAM bounce, all on the gpsimd queue.
  - attention outputs stay in SBUF as 12 [128,L] bf16 head-pair tiles
    consumed directly by the O projection; output shipped bf16.
  - the q/k "+eps" of the reference is dropped (~1e-7 relative effect); the
    attention-level eps is kept via the cs rank-1 correction and the
    rowsum + L*eps denominator.
"""

import os
import sys

sys.path.insert(0, "/opt/trn_rl_repo")

import numpy as np

B, L, F, H, D = 16, 1025, 768, 12, 64
NB = 32
EPS = 1e-8
LP = 1152           # padded token count (9 * 128)
NKB = 9             # k blocks of 128
QM = 1024           # main q width (q tail = 1 col, index 1024)
FA = F + 1          # augmented contraction (ones row)
NCORES = 8
BPC = B // NCORES   # batches per core

_PROG = None


def _build_program():
    import concourse.bass as bass
    import concourse.tile as tile
    from concourse import mybir

    f32 = mybir.dt.float32
    bf16 = mybir.dt.bfloat16
    AF = mybir.ActivationFunctionType

    nc = bass.Bass()

    xaP = nc.declare_dram_parameter("xaP", [BPC, 128, 6, LP], bf16, isOutput=False)
    wqP = nc.declare_dram_parameter("wqP", [6, 128, 6, 128], bf16, isOutput=False)
    wkP = nc.declare_dram_parameter("wkP", [6, 128, 6, 128], bf16, isOutput=False)
    wvP = nc.declare_dram_parameter("wvP", [2, 128, 7, 390], bf16, isOutput=False)
    woP = nc.declare_dram_parameter("woP", [128, 6, F], bf16, isOutput=False)
    boP = nc.declare_dram_parameter("boP", [128, 6], f32, isOutput=False)
    bqkP = nc.declare_dram_parameter("bqkP", [128, 12], f32, isOutput=False)
    csP = nc.declare_dram_parameter("csP", [65, 24], f32, isOutput=False)
    maskP = nc.declare_dram_parameter(
        "maskP", [H, 128, NKB, QM], bf16, isOutput=False
    )
    mask_tail = nc.declare_dram_parameter(
        "maskT_tail", [H, 128, NKB], bf16, isOutput=False
    )
    yT = nc.declare_dram_parameter("yT", [BPC, 128, 6, L], bf16, isOutput=True)

    # rows padded to LP so [128, 9] partition-transposed reads/writes of the
    # 1025 live elements stay in-bounds
    rr_dram = nc.dram_tensor("rr_dram", [8, LP], f32)
    rsum_dram = nc.dram_tensor("rsum_dram", [8, LP], f32)

    with tile.TileContext(nc) as tc:
        from contextlib import ExitStack

        with ExitStack() as octx:
            consts = octx.enter_context(tc.tile_pool(name="consts", bufs=1))
            # attention outputs, SBUF-resident across phases: 12 tiles
            # [128, L] bf16, one per (batch, head-pair); rows 0:64 = even
            # head, 64:128 = odd head of the pair
            ot_pool = octx.enter_context(tc.tile_pool(name="ot", bufs=2 * 6))
            wo_pool = octx.enter_context(tc.tile_pool(name="wo", bufs=1))
            ctx = octx.enter_context(ExitStack())
            xa_pool = ctx.enter_context(tc.tile_pool(name="xa", bufs=2))
            wqk_pool = ctx.enter_context(tc.tile_pool(name="wqk", bufs=2))
            wv_pool = ctx.enter_context(tc.tile_pool(name="wv", bufs=2))
            qkt_pool = ctx.enter_context(tc.tile_pool(name="qkt", bufs=2))
            vaug_pool = ctx.enter_context(tc.tile_pool(name="vaug", bufs=4))
            mask_pool = ctx.enter_context(tc.tile_pool(name="mask", bufs=2))
            mtail_pool = ctx.enter_context(tc.tile_pool(name="mtail", bufs=2))
            mt_pool = ctx.enter_context(tc.tile_pool(name="mt", bufs=4))
            std_pool = ctx.enter_context(tc.tile_pool(name="std", bufs=2))
            mttail_pool = ctx.enter_context(tc.tile_pool(name="mttail", bufs=2))
            rs_pool = ctx.enter_context(tc.tile_pool(name="rs", bufs=1))
            rrb_pool = ctx.enter_context(tc.tile_pool(name="rrb", bufs=3))
            avsb_pool = ctx.enter_context(tc.tile_pool(name="avsb", bufs=4))

            # flex pool: [128,512] tiles time-shared between projection psums
            # (2-deep so the activation drain doesn't stall the next matmul
            # group) and the per-head tail psum (stail+avt live in a slice)
            ps_flex = ctx.enter_context(
                tc.tile_pool(name="ps_flex", bufs=2, space="PSUM")
            )
            ps_s = ctx.enter_context(tc.tile_pool(name="ps_s", bufs=2, space="PSUM"))
            ps_av = ctx.enter_context(tc.tile_pool(name="ps_av", bufs=1, space="PSUM"))

            # three DMA-issue queues: sync carries x/weights/outputs, the
            # scalar queue carries the big mask transfers, gpsimd carries
            # the normalize path + upfront small loads
            dma = nc.sync
            dma2 = nc.gpsimd
            dma3 = nc.scalar

            # constants
            ones_row = consts.tile([1, LP], bf16)
            nc.vector.memset(ones_row[:, 0:L], 1.0)
            nc.vector.memset(ones_row[:, L:LP], 0.0)
            bq_all = consts.tile([128, 12], f32, name="bq_all")
            dma2.dma_start(out=bq_all, in_=bqkP[:, :])
            cs_all = consts.tile([65, 24], f32, name="cs_all")
            dma2.dma_start(out=cs_all, in_=csP[:, :])
            bo_sb = consts.tile([128, 6], f32, name="bo_sb")
            dma2.dma_start(out=bo_sb, in_=boP[:, :])

            ot_pairs = {}
            for b in range(BPC):
                for pair in range(6):
                    ot_pairs[(b, pair)] = ot_pool.tile(
                        [128, L], bf16, tag="ot", name="ot_pair"
                    )

            # ---- persistent x in SBUF: one [128, 6, LP] tile per batch ---
            # per-chunk DMAs so the first V-proj matmul starts as soon as
            # chunk 0 lands instead of waiting for the whole 1.8MB tile
            xa_t = {}
            for b in range(BPC):
                t = xa_pool.tile([128, 6, LP], bf16, tag="xa", name="xa_tile")
                for c in range(6):
                    dma.dma_start(
                        out=t[:, c : c + 1, :], in_=xaP[b, :, c : c + 1, :]
                    )
                xa_t[b] = t

            # output-projection weights, prefetched so the O phase starts
            # without a DMA stall
            wo_sb = wo_pool.tile([128, 6, F], bf16, name="wo_sb")
            dma2.dma_start(out=wo_sb, in_=woP[:, :, :])

            # q sub-tiles for projections (moving dim <= 512); only token
            # 1024 of the padded tail is real
            qsubs = [(0, 512), (512, 512), (1024, 1)]
            # attention q tiling: main [0,1024) in 2 psum-bank halves + tail col
            def st_slices():
                return [(0, 512), (512, 512)]

            # ---- v projections, per 3-pair group ------------------------
            # wv columns are grouped per head: h*65 + (0..63 -> wv, 64 -> ones)
            vaug = {}      # (b, g) -> [128, NKB, 390]

            wv_tiles = {}

            def emit_vproj(g, b):
                if g not in wv_tiles:
                    wv_sb = wv_pool.tile([128, 7, 390], bf16, tag="wv")
                    for c in range(7):
                        dma3.dma_start(
                            out=wv_sb[:, c : c + 1, :], in_=wvP[g, :, c : c + 1, :]
                        )
                    wv_tiles[g] = wv_sb
                wv_sb = wv_tiles[g]
                va = vaug_pool.tile([128, NKB, 390], bf16, tag="vaug")
                for tb in range(NKB):
                    ps = ps_flex.tile([128, 512], f32, tag="flex", name="ps_v")
                    for c in range(6):
                        nc.tensor.matmul(
                            ps[:, 0:390],
                            xa_t[b][:, c, tb * 128 : (tb + 1) * 128],
                            wv_sb[:, c, :],
                            start=(c == 0),
                            stop=False,
                        )
                    nc.tensor.matmul(
                        ps[:, 0:390],
                        ones_row[:, tb * 128 : (tb + 1) * 128],
                        wv_sb[0:1, 6, :],
                        start=False,
                        stop=True,
                    )
                    nc.scalar.activation(va[:, tb, :], ps[:, 0:390], AF.Copy)
                vaug[(b, g)] = va

            # ---- main loop over head pairs ------------------------------
            for pair in range(6):
                g = pair // 3

                # qT/kT projections for this pair, both batches
                wq_sb = wqk_pool.tile([128, 6, 128], bf16, tag="wq")
                wk_sb = wqk_pool.tile([128, 6, 128], bf16, tag="wk")
                dma.dma_start(out=wq_sb, in_=wqP[pair])
                dma.dma_start(out=wk_sb, in_=wkP[pair])

                qT = {}
                kT = {}

                def qk_proj(b):
                    qt = qkt_pool.tile([128, LP], bf16, tag="qT")
                    kt = qkt_pool.tile([128, LP], bf16, tag="kT")
                    # k-pad columns are read by the j=8 S matmul (masked to
                    # zero afterwards) - keep them finite.  The kT tag
                    # rotates between 2 physical buffers and nothing else
                    # ever writes the pad columns, so zeroing the first two
                    # allocations zeroes them for the whole kernel.
                    if pair == 0:
                        nc.vector.memset(kt[:, L:LP], 0.0)
                    for (dst, w_sb, scl, bi) in (
                        (qt, wq_sb, 0.125, 0),
                        (kt, wk_sb, 1.0, 1),
                    ):
                        for (q0, qw) in qsubs:
                            psq = ps_flex.tile(
                                [128, 512], f32, tag="flex", name="ps_qk"
                            )
                            for c in range(6):
                                nc.tensor.matmul(
                                    psq[:, 0:qw],
                                    w_sb[:, c, :],
                                    xa_t[b][:, c, q0 : q0 + qw],
                                    start=(c == 0), stop=(c == 5),
                                )
                            # relu(scale*xw + scale*b); the reference's +eps
                            # here is dropped (~1e-7 relative effect)
                            nc.scalar.activation(
                                dst[:, q0 : q0 + qw], psq[:, 0:qw], AF.Relu,
                                scale=scl,
                                bias=bq_all[:, 2 * pair + bi : 2 * pair + bi + 1],
                            )
                    qT[b] = qt
                    kT[b] = kt

                if pair == 0:
                    # interleave so b=0 compute overlaps the arrival of
                    # batch 1's x DMA at kernel start
                    emit_vproj(g, 0)
                    qk_proj(0)
                    emit_vproj(g, 1)
                    qk_proj(1)
                else:
                    if pair % 3 == 0:
                        emit_vproj(g, 0)
                        emit_vproj(g, 1)
                    qk_proj(0)
                    qk_proj(1)

                av_sbs = {}
                for hh in range(2):
                    h = pair * 2 + hh
                    r0 = hh * 64
                    # mask tile for this head (shared across batches)
                    mk = mask_pool.tile(
                        [128, NKB, QM], bf16, tag="mask", name="mask_tile"
                    )
                    dma3.dma_start(out=mk, in_=maskP[h])
                    mkt = mtail_pool.tile([128, NKB], bf16, tag="mtail")
                    dma3.dma_start(out=mkt, in_=mask_tail[h])

                    for b in range(BPC):
                        va = vaug[(b, pair // 3)]
                        vc0 = (pair % 3) * 130 + hh * 65

                        av = ps_av.tile([65, QM], f32, tag="ps_av")
                        ptl = ps_flex.tile(
                            [128, 512], f32, tag="flex", name="ps_tails"
                        )
                        stail = ptl[:, 0:NKB]
                        avt = ptl[0:65, NKB : NKB + 1]
                        mtt = mttail_pool.tile([128, NKB], bf16, tag="mttail")

                        # software-pipelined at depth 2: AV_j issues after
                        # S_{j+2}, so the PE never waits on the scalar
                        # drain + DVE 2x-mode mask-multiply chain
                        def emit_s(j):
                            lhs_k = kT[b][r0 : r0 + 64, j * 128 : (j + 1) * 128]
                            st = ps_s.tile([128, QM], f32, tag="ps_s")
                            for (q0, qw) in st_slices():
                                nc.tensor.matmul(
                                    st[:, q0 : q0 + qw],
                                    lhs_k,
                                    qT[b][r0 : r0 + 64, q0 : q0 + qw],
                                    start=True, stop=True,
                                )
                            # tail column q=1024 (shares the kT weights)
                            nc.tensor.matmul(
                                stail[:, j : j + 1],
                                lhs_k,
                                qT[b][r0 : r0 + 64, QM : QM + 1],
                                start=True, stop=True,
                            )
                            # masked scores -> bf16.  Alternate per j: even
                            # j drains to bf16 on the scalar engine so the
                            # DVE multiply runs in 2x mode; odd j multiplies
                            # straight from PSUM at 1x.  This splits the
                            # ~1.1us/j chain across both engines so neither
                            # becomes the S-loop rate limiter.
                            mt = mt_pool.tile([128, QM], bf16, tag="mt")
                            if j % 2 == 0:
                                std = std_pool.tile([128, QM], bf16, tag="std")
                                nc.scalar.activation(std, st, AF.Copy)
                                nc.vector.tensor_mul(mt, std, mk[:, j, :])
                            else:
                                nc.vector.tensor_mul(mt, st, mk[:, j, :])
                            return mt

                        def emit_av(j, mt):
                            # AV accumulation (row 64 = rowsum via ones col)
                            for (q0, qw) in st_slices():
                                nc.tensor.matmul(
                                    av[:, q0 : q0 + qw],
                                    va[:, j, vc0 : vc0 + 65],
                                    mt[:, q0 : q0 + qw],
                                    start=(j == 0), stop=(j == NKB - 1),
                                )

                        mts = [emit_s(0), emit_s(1)]
                        for j in range(2, NKB):
                            mts.append(emit_s(j))
                            emit_av(j - 2, mts[j - 2])
                        emit_av(NKB - 2, mts[NKB - 2])
                        emit_av(NKB - 1, mts[NKB - 1])

                        # tail: masked scores + AV
                        nc.vector.tensor_mul(mtt, stail, mkt)
                        for j in range(NKB):
                            nc.tensor.matmul(
                                avt,
                                va[:, j, vc0 : vc0 + 65],
                                mtt[:, j : j + 1],
                                start=(j == 0), stop=(j == NKB - 1),
                            )

                        # drain AV psum to SBUF (frees the banks for the
                        # next head while the normalize chain runs)
                        av_sb = avsb_pool.tile([65, L], f32, tag="avsb")
                        nc.scalar.activation(av_sb[:, 0:QM], av, AF.Copy)
                        nc.scalar.activation(av_sb[:, QM : QM + 1], avt, AF.Copy)
                        av_sbs[(hh, b)] = av_sb

                # ---- batched normalization for the pair's 4 (hh, b) -----
                # gather rowsum rows TRANSPOSED across partitions so the
                # reciprocal runs ~36 elems/lane (~0.3us) instead of 1025
                # serial elems on 4 lanes (~6.5us blocking the DVE queue)
                def normalize(combos, slot0):
                    n = len(combos)
                    rs = rs_pool.tile([4, L], f32, tag="rs")
                    for idx, (hh, b) in enumerate(combos):
                        dma2.dma_start(
                            out=rs[idx : idx + 1, :],
                            in_=av_sbs[(hh, b)][64:65, :],
                        )
                    nc.vector.tensor_scalar_add(
                        rs[0:n], rs[0:n], float(L) * EPS
                    )
                    rr = rs_pool.tile([4, L], f32, tag="rr")
                    nc.vector.reciprocal(rr[0:n], rs[0:n])
                    dma2.dma_start(
                        out=rr_dram[slot0 : slot0 + n, 0:L], in_=rr[0:n]
                    )
                    for idx, (hh, b) in enumerate(combos):
                        rr_slot = rr_dram[slot0 + idx, 0:L]
                        rr_bcast_src = bass.AP(
                            tensor=rr_slot.tensor,
                            offset=rr_slot.offset,
                            ap=[[0, 64]] + list(rr_slot.ap),
                        )
                        rrb = rrb_pool.tile([64, L], f32, tag="rrb")
                        dma2.dma_start(out=rrb, in_=rr_bcast_src)
                        hg = (pair % 3) * 2 + hh
                        ci = b * 12 + g * 6 + hg
                        r0h = hh * 64
                        nc.vector.scalar_tensor_tensor(
                            ot_pairs[(b, pair)][r0h : r0h + 64, :],
                            av_sbs[(hh, b)][0:64, :],
                            cs_all[0:64, ci : ci + 1],
                            rrb,
                            op0=mybir.AluOpType.add,
                            op1=mybir.AluOpType.mult,
                        )

                if pair < 5:
                    normalize(
                        [(hh, b) for hh in range(2) for b in range(BPC)],
                        (pair % 2) * 4,
                    )
                else:
                    # last pair: per-batch so the O projection of b=0 isn't
                    # gated on b=1's normalize chain
                    normalize([(0, 0), (1, 0)], 4)
                    normalize([(0, 1), (1, 1)], 6)

            # ---- output projection: yT = wo^T @ O^T + bo ----------------
            ctx.close()
            ctx = octx.enter_context(ExitStack())
            y_pool = ctx.enter_context(tc.tile_pool(name="y", bufs=3))
            ps_y = ctx.enter_context(tc.tile_pool(name="ps_y", bufs=2, space="PSUM"))

            oq_tiles = [(0, 512), (512, 512), (1024, 1)]
            for b in range(BPC):
                for fc in range(6):
                    y_tile = y_pool.tile([128, L], bf16, tag="y", name="y_tile")
                    for (q0, qw) in oq_tiles:
                        psy = ps_y.tile([128, 512], f32, tag="ps_y")
                        for hc in range(6):
                            nc.tensor.matmul(
                                psy[:, 0:qw],
                                wo_sb[:, hc, fc * 128 : (fc + 1) * 128],
                                ot_pairs[(b, hc)][:, q0 : q0 + qw],
                                start=(hc == 0), stop=(hc == 5),
                            )
                        # drain with bo fused as the per-partition bias
                        nc.scalar.activation(
                            y_tile[:, q0 : q0 + qw], psy[:, 0:qw],
                            AF.Identity, bias=bo_sb[:, fc : fc + 1],
                        )
                    dma.dma_start(out=yT[b][:, fc, :], in_=y_tile)

    _split_matmul_waits(nc)
    return nc


def _split_matmul_waits(nc):
    """Walrus TPB instruction structs encode a limited number of sync waits
    (the fp32 LDWEIGHTS+MATMUL pair can take none beyond its update).  Hoist
    excess waits onto same-engine NoOps inserted just before each
    instruction."""
    import bass_rust
    from concourse import mybir

    n = 0
    for f in nc.m.functions:
        for blk in f.blocks:
            insts = blk.instructions
            out = []
            for inst in insts:
                si = inst.sync_info
                tname = type(inst).__name__
                if si is not None and len(si.on_wait) > 0 and tname != "InstISA":
                    cap = 0 if tname == "InstMatmult" else 1
                    waits = list(si.on_wait)
                    if len(waits) > cap:
                        hoist = waits[: len(waits) - cap]
                        keep = waits[len(waits) - cap :]
                        for w in hoist:
                            nop = mybir.InstNoOp(
                                name=f"I-mmw-{n}", ins=[], outs=[]
                            )
                            n += 1
                            nop.engine = inst.engine
                            nop.sync_info = bass_rust.SyncInfo(
                                on_wait=[w], on_update=[]
                            )
                            out.append(nop)
                        inst.sync_info = bass_rust.SyncInfo(
                            on_wait=keep, on_update=list(si.on_update)
                        )
                out.append(inst)
            insts[:] = out
    return n


def _dist_index():
    gi = np.arange(NB)
    gj = np.arange(NB)
    idx = (
        (gi[:, None, None, None] - gi[None, None, :, None] + NB) * 2 * NB
        + gj[None, :, None, None]
        - gj[None, None, None, :]
        + NB
    )
    return idx.reshape(-1).astype(np.int32)


def _host_prep(x, wq, bq, wk, bk, wv, bv, wo, bo, toeplitz_params):
    import ml_dtypes

    f4 = np.float32
    bf = ml_dtypes.bfloat16
    x = np.asarray(x, f4)
    L0 = NB * NB

    # x, transposed to [F, L], padded to LP, packed [128, 6, LP]
    xs = np.transpose(x, (0, 2, 1))  # [B, F, L]
    xaP = np.zeros((B, 128, 6, LP), bf)
    xaP[:, :, :, :L] = xs.reshape(B, 6, 128, L).transpose(0, 2, 1, 3).astype(bf)

    wq_flat = np.asarray(wq, f4).reshape(F, F)
    wk_flat = np.asarray(wk, f4).reshape(F, F)
    wqP = np.ascontiguousarray(
        wq_flat.reshape(6, 128, 6, 128).transpose(2, 1, 0, 3).astype(bf)
    )
    wkP = np.ascontiguousarray(
        wk_flat.reshape(6, 128, 6, 128).transpose(2, 1, 0, 3).astype(bf)
    )

    wvr = np.asarray(wv, f4)
    bvr = np.asarray(bv, f4)
    wv_aug = np.zeros((FA, H * 65), f4)
    for h in range(H):
        wv_aug[:F, h * 65 : h * 65 + 64] = wvr[:, h, :]
        wv_aug[F, h * 65 : h * 65 + 64] = bvr[h]
        wv_aug[F, h * 65 + 64] = 1.0
    wvP = np.zeros((2, 128, 7, 390), bf)
    wvP[:, :, :6, :] = (
        wv_aug[:F].reshape(6, 128, 2, 390).transpose(2, 1, 0, 3).astype(bf)
    )
    wvP[:, 0, 6, :] = wv_aug[F].reshape(2, 390).astype(bf)

    wo_flat = np.asarray(wo, f4).reshape(H * D, F)
    woP = np.ascontiguousarray(
        wo_flat.reshape(6, 128, F).transpose(1, 0, 2).astype(bf)
    )
    boP = np.ascontiguousarray(np.asarray(bo, f4).reshape(6, 128).T)

    bqs = (np.asarray(bq, f4).reshape(F) * 0.125).reshape(6, 128)
    bks = np.asarray(bk, f4).reshape(F).reshape(6, 128)
    bqkP = np.zeros((128, 12), f4)
    bqkP[:, 0::2] = bqs.T
    bqkP[:, 1::2] = bks.T

    # gathered |toeplitz| mask, padded (CLS row/col of ones), transposed,
    # k padded to 1152 with zeros, packed [H, 128, NKB, QM]
    tp = np.asarray(toeplitz_params, f4)
    tm = np.abs(tp[:, _dist_index()]).reshape(H, L0, L0)
    tm_full = np.ones((H, L, L), f4)
    tm_full[:, 1:, 1:] = tm
    maskT = np.zeros((H, LP, L), bf)
    maskT[:, :L, :] = np.transpose(tm_full, (0, 2, 1)).astype(bf)
    maskP = np.ascontiguousarray(
        maskT[:, :, :QM].reshape(H, NKB, 128, QM).transpose(0, 2, 1, 3)
    )
    maskT_tail = np.ascontiguousarray(
        maskT[:, :, QM].reshape(H, NKB, 128).transpose(0, 2, 1)
    )

    xsum = x.sum(axis=1)  # [B, F]
    cs = np.einsum("bf,fhd->bhd", xsum, wvr) + L * bvr[None]  # [B, H, 64]
    cs_full = np.concatenate(
        [cs, np.full((B, H, 1), float(L), np.float32)], axis=2
    ) * np.float32(EPS)  # [B, H, 65]

    shared = dict(
        bqkP=bqkP,
        wqP=wqP,
        wkP=wkP,
        wvP=wvP,
        woP=woP,
        boP=boP,
        maskP=maskP,
        maskT_tail=maskT_tail,
    )
    in_maps = []
    for c in range(NCORES):
        m = dict(shared)
        m["xaP"] = np.ascontiguousarray(xaP[c * BPC : (c + 1) * BPC])
        csP = np.zeros((65, 24), f4)
        for b in range(BPC):
            for g in range(2):
                for hg in range(6):
                    csP[:, b * 12 + g * 6 + hg] = cs_full[
                        c * BPC + b, 6 * g + hg, :
                    ]
        m["csP"] = csP
        in_maps.append(m)
    return in_maps


def _get_program():
    global _PROG
    if _PROG is None:
        _PROG = _build_program()
    return _PROG


def run(trace=False, **inputs):
    from concourse.bass_utils import run_bass_kernel_spmd

    nc = _get_program()
    in_maps = _host_prep(**inputs)
    res = run_bass_kernel_spmd(nc, in_maps, list(range(NCORES)), trace=trace)
    outs = []
    for c in range(NCORES):
        yt = np.asarray(res.results[c]["yT"], dtype=np.float32)  # [BPC,128,6,L]
        # y[b, l, fc*128 + p] = yt[b, p, fc, l]
        outs.append(yt.transpose(0, 3, 2, 1).reshape(BPC, L, F))
    y = np.concatenate(outs, axis=0).astype(np.float32)
    return y, res


def kernel(**inputs):
    y, _ = run(trace=False, **inputs)
    return y
